# revision 12
# baseline (speedup 1.0000x reference)
"""GQA flash attention (B=2, S=2048, DM=1024, H=16, Hkv=4, HD=64) on 8 TRN2
NeuronCores.

Sharding: core i handles (batch b = i//4, kv-group g = i%4): its 4 query
heads + 1 KV head. Each core computes x@Wq/Wk/Wv for its slice, continuous
2D-RoPE, full (non-causal) softmax attention, and its partial o_proj
contribution y_g^T = Wo_g^T @ O_g^T; the host sums the 4 partials per batch.

Design notes (vs the f32r baseline):
- Everything on the PE runs in bf16 (same PE rate as f32r at 512-wide
  tiles, half the SBUF/DMA footprint, 2x DVE throughput on the all-bf16
  RoPE elementwise ops). fp8 was measured and rejected: attention output
  over near-uniform softmax weights is itself a weighted average, so fp8
  quantization noise on P or V passes through at full ~3% relative.
- P*V uses a 65-column V tile [V(64) | ones]: each matmul yields the 64
  O-dims (partitions 0:64) and the softmax denominator (partition 64) in
  one pass, both quadrant-aligned for the DVE drains.
- K is kept once in bf16, duplicated across both partition halves of one
  tile, so each head-half QK matmul contracts d=64 with matching bases.
- The attention phase is a flat software-pipelined stream over
  (qt, head-pair, key-tile) steps: the QK matmuls + exp of step s are
  emitted BEFORE the P*V matmuls of step s-1, so the ACT exp stream
  (the ~147us/iteration critical path: 128 exps x (1024+352)cyc) never
  waits on PE at block boundaries.
- The steady-state loop body holds TWO logical iterations with parity-
  swapped x/Q/K/V buffers; projections for iteration i+1 are emitted as
  small work units interleaved into iteration i's attention stream, so
  the PE stays continuously busy (full pstate) and ACT never idles
  during projections. o_proj of each query tile, and the softmax
  normalization (reciprocal + broadcast matmul + scale), are likewise
  deferred into later steps, carried across phase and loop boundaries
  via fixed 2-deep tile rings.
- ACT does exp only; all PSUM drains are on DVE.
"""
import sys
sys.path.insert(0, "/opt/trn_rl_repo")
import numpy as np
import ml_dtypes

B, S, DM = 2, 2048, 1024
H, HKV, HD = 16, 4, 64
THETA = 10000.0
NCORE = 8
KT = DM // 128    # 8  contraction tiles for projections
ST = S // 512     # 4  query tiles
NKT = S // 128    # 16 key tiles

# per-head d permutation: evens of x-half, evens of y-half, odds of x, odds of y
_PE = np.concatenate([np.arange(0, 32, 2), np.arange(32, 64, 2)])
_PO = _PE + 1
PERM64 = np.concatenate([_PE, _PO])  # [64]

# sel rows -> og partition halves: bc[j] = rc[0] for j<64 else rc[1]
_SEL = np.zeros((2, 128), np.float32)
_SEL[0, 0:64] = 1.0
_SEL[1, 64:128] = 1.0

# rotate-half as a one-hot matrix: row i of (PERMM.T @ t) = t[swap(i)],
# swap exchanges 32-partition blocks (0:32<->32:64, 64:96<->96:128).
_SWAP = np.arange(128)
_SWAP = np.concatenate([_SWAP[32:64], _SWAP[0:32], _SWAP[96:128], _SWAP[64:96]])
_PERMM = np.zeros((128, 128), np.float32)
for _j in range(128):
    _PERMM[_SWAP[_j], _j] = 1.0

_prog_cache = {}


def _build_program(repeat=1):
    import concourse.bacc as bacc
    import concourse.tile as tile
    from concourse import mybir
    from concourse.masks import make_identity
    from contextlib import ExitStack

    f32 = mybir.dt.float32
    f32r = mybir.dt.float32r
    bf16 = mybir.dt.bfloat16
    Exp = mybir.ActivationFunctionType.Exp

    assert repeat == 1 or repeat % 2 == 0

    nc = bacc.Bacc(None, target_bir_lowering=False)
    xT = nc.dram_tensor("xT", [DM, S], bf16, kind="ExternalInput")
    wq = nc.dram_tensor("wq", [DM, 256], bf16, kind="ExternalInput")
    wkv = nc.dram_tensor("wkv", [DM, 128], bf16, kind="ExternalInput")
    wo = nc.dram_tensor("wo", [256, DM], bf16, kind="ExternalInput")
    tqc = nc.dram_tensor("tqc", [128, S], bf16, kind="ExternalInput")
    tqs = nc.dram_tensor("tqs", [128, S], bf16, kind="ExternalInput")
    tkc = nc.dram_tensor("tkc", [64, S], bf16, kind="ExternalInput")
    tks = nc.dram_tensor("tks", [64, S], bf16, kind="ExternalInput")
    seld = nc.dram_tensor("seld", [2, 128], f32r, kind="ExternalInput")
    permd = nc.dram_tensor("permd", [128, 128], bf16, kind="ExternalInput")
    yT = nc.dram_tensor("yT", [DM, S], f32, kind="ExternalOutput")

    xT_t = xT[:].rearrange("(kt p) s -> p kt s", p=128)
    wq_t = wq[:].rearrange("(kt p) m -> p kt m", p=128)
    wkv_t = wkv[:].rearrange("(kt p) m -> p kt m", p=128)
    wo_t = wo[:].rearrange("(kt p) e -> p kt e", p=128)
    yT_t = yT[:].rearrange("(mt p) s -> p mt s", p=128)

    nparity = 1 if repeat == 1 else 2
    QSL = [slice(qt * 512, (qt + 1) * 512) for qt in range(ST)]

    with ExitStack() as ctx:
        tc = ctx.enter_context(tile.TileContext(nc))
        persist = ctx.enter_context(tc.tile_pool(name="persist", bufs=1))

        # ---- loop-invariant constants / weights / tables ----
        x_sb = [persist.tile([128, KT, S], bf16, name=f"x_sb{p}")
                for p in range(nparity)]
        QA = [persist.tile([128, S], bf16, name=f"QA{p}")
              for p in range(nparity)]
        QB = [persist.tile([128, S], bf16, name=f"QB{p}")
              for p in range(nparity)]
        KB = [persist.tile([128, S], bf16, name=f"KB{p}")
              for p in range(nparity)]
        V_sb = [persist.tile([128, NKT, 66], bf16, name=f"V_sb{p}")
                for p in range(nparity)]
        wq_sb = persist.tile([128, KT, 256], bf16, name="wq_sb")
        wkv_sb = persist.tile([128, KT, 128], bf16, name="wkv_sb")
        wo_sb = persist.tile([128, 2, DM], bf16, name="wo_sb")
        tqc_sb = persist.tile([128, S], bf16, name="tqc_sb")
        tqs_sb = persist.tile([128, S], bf16, name="tqs_sb")
        tkc_sb = persist.tile([64, S], bf16, name="tkc_sb")
        tks_sb = persist.tile([64, S], bf16, name="tks_sb")
        perm = persist.tile([128, 128], bf16, name="perm")
        sel = persist.tile([2, 128], f32r, name="sel")
        ident = persist.tile([128, 64], bf16, name="ident")
        # carried rings (attention output + denominators survive across
        # phase/loop boundaries; 2-deep is enough at the consumption lag)
        og_ring = [persist.tile([128, 2, 512], bf16, name=f"og{r}")
                   for r in range(2)]
        dgq_ring = [persist.tile([2, 512], f32, name=f"dgq{r}")
                    for r in range(2)]

        # ---- working pools (shared by preamble proj and loop body) ----
        ps_sc = ctx.enter_context(
            tc.tile_pool(name="ps_sc", bufs=2, space="PSUM"))
        ps_acc = ctx.enter_context(
            tc.tile_pool(name="ps_acc", bufs=1, space="PSUM"))
        ps_sm = ctx.enter_context(
            tc.tile_pool(name="ps_sm", bufs=2, space="PSUM"))
        rawp = ctx.enter_context(tc.tile_pool(name="rawp", bufs=4))
        ptp = ctx.enter_context(tc.tile_pool(name="ptp", bufs=4))
        oxp = ctx.enter_context(tc.tile_pool(name="oxp", bufs=3))
        ystp = ctx.enter_context(tc.tile_pool(name="ystp", bufs=4))

        onesf = rawp.tile([128, NKT], f32, name="onesf", tag="ones", bufs=1)
        nc.vector.memset(onesf, 1.0)
        for r in range(2):
            nc.vector.memset(og_ring[r].bitcast(f32), 0.0)
            nc.vector.memset(dgq_ring[r], 1.0)
        for p in range(nparity):
            nc.vector.tensor_copy(V_sb[p][:, :, 64], onesf)
            nc.vector.tensor_copy(V_sb[p][:, :, 65], onesf)
        nc.sync.dma_start(sel, seld[:])
        nc.sync.dma_start(perm, permd[:])
        make_identity(nc, ident[64:128, :])
        nc.sync.dma_start(wo_sb[:, 0, :], wo_t[:, 0, :])
        nc.sync.dma_start(wo_sb[:, 1, :], wo_t[:, 1, :])
        nc.sync.dma_start(tqc_sb, tqc[:])
        nc.sync.dma_start(tqs_sb, tqs[:])
        nc.sync.dma_start(tkc_sb, tkc[:])
        nc.sync.dma_start(tks_sb, tks[:])
        for kt in range(KT):
            nc.sync.dma_start(wq_sb[:, kt, :], wq_t[:, kt, :])
            nc.sync.dma_start(wkv_sb[:, kt, :], wkv_t[:, kt, :])
            for p in range(nparity):
                nc.sync.dma_start(x_sb[p][:, kt, :], xT_t[:, kt, :])

        # ---------- projection helpers ----------
        def proj_q_unit(p, st, mt):
            """x@Wq (one head-pair column block) + RoPE for one seq tile."""
            xs = x_sb[p]
            qdst = QA[p] if mt == 0 else QB[p]
            sl = slice(st * 512, (st + 1) * 512)
            pq = ps_sm.tile([128, 512], f32, name="pq", tag="sm")
            for kt in range(KT):
                nc.tensor.matmul(
                    pq, lhsT=wq_sb[:, kt, mt * 128:(mt + 1) * 128],
                    rhs=xs[:, kt, sl], start=(kt == 0), stop=(kt == KT - 1))
            qraw = rawp.tile([128, 512], bf16, name="qraw", tag="raw")
            nc.vector.tensor_copy(qraw, pq)
            tmp = rawp.tile([128, 512], bf16, name="tmp", tag="tmp")
            # rotate-half = 32-partition block swap (copies), then one mul
            for (d, sq) in ((0, 32), (32, 0), (64, 96), (96, 64)):
                nc.vector.tensor_copy(tmp[d:d + 32, :], qraw[sq:sq + 32, :])
            nc.vector.tensor_mul(tmp, tmp, tqs_sb[:, sl])
            nc.gpsimd.tensor_mul(qdst[:, sl], qraw, tqc_sb[:, sl])
            nc.gpsimd.tensor_add(qdst[:, sl], qdst[:, sl], tmp)

        def proj_kv_unit(p, st):
            """x@Wkv for one seq tile: K rope into KB, V^T into V_sb."""
            xs, kb, vsb = x_sb[p], KB[p], V_sb[p]
            sl = slice(st * 512, (st + 1) * 512)
            pkv = ps_sm.tile([128, 512], f32, name="pkv", tag="sm")
            for kt in range(KT):
                nc.tensor.matmul(
                    pkv, lhsT=wkv_sb[:, kt, :], rhs=xs[:, kt, sl],
                    start=(kt == 0), stop=(kt == KT - 1))
            kvraw = rawp.tile([128, 512], bf16, name="kvraw", tag="raw")
            nc.vector.tensor_copy(kvraw, pkv)
            tmpk = rawp.tile([128, 512], bf16, name="tmpk", tag="tmp")
            for (d, sq) in ((0, 32), (32, 0)):
                nc.vector.tensor_copy(tmpk[d:d + 32, :], kvraw[sq:sq + 32, :])
            nc.vector.tensor_mul(tmpk[0:64, :], tmpk[0:64, :], tks_sb[:, sl])
            nc.vector.tensor_mul(kb[0:64, sl], kvraw[0:64, :], tkc_sb[:, sl])
            nc.vector.tensor_add(kb[0:64, sl], kb[0:64, sl], tmpk[0:64, :])
            nc.sync.dma_start(kb[64:128, sl], kb[0:64, sl])
            for j in range(4):
                kt_i = st * 4 + j
                pv = ps_sm.tile([128, 64], bf16, name="pv", tag="sm")
                nc.tensor.transpose(
                    pv, kvraw[64:128, j * 128:(j + 1) * 128], ident[64:128, :])
                nc.vector.tensor_copy(vsb[:, kt_i, 0:64], pv)

        def proj_units(p):
            for st in range(ST):
                yield lambda st=st: proj_kv_unit(p, st)
                yield lambda st=st: proj_q_unit(p, st, 0)
                yield lambda st=st: proj_q_unit(p, st, 1)

        def emit_proj_block(p):
            for u in proj_units(p):
                u()

        def emit_oproj(og_prev, qsl_prev, mt):
            yp = ps_sm.tile([128, 512], f32, name="yp", tag="sm")
            for k2 in range(2):
                nc.tensor.matmul(
                    yp, lhsT=wo_sb[:, k2, mt * 128:(mt + 1) * 128],
                    rhs=og_prev[:, k2, :], start=(k2 == 0), stop=(k2 == 1))
            yst = ystp.tile([128, 512], f32, name="yst")
            if mt % 2 == 0:
                nc.vector.tensor_copy(yst, yp)
            else:
                nc.scalar.copy(yst, yp)
            nc.sync.dma_start(yT_t[:, mt, qsl_prev], yst)

        def flush_norm(stt):
            while stt["norm"]:
                og_n, pss_n, dgq = stt["norm"].pop(0)
                rcf = oxp.tile([2, 512], f32, name="rcf", tag="rcf")
                nc.vector.reciprocal_approx_fast(out=rcf, in_=dgq)
                rc2 = oxp.tile([2, 512], f32r, name="rc2", tag="rc2")
                nc.vector.tensor_copy(rc2, rcf)
                bc = ps_sm.tile([128, 512], f32, name="bc", tag="sm")
                nc.tensor.matmul(bc, lhsT=sel, rhs=rc2, start=True, stop=True)
                bcs = oxp.tile([128, 512], bf16, name="bcs", tag="bcs")
                nc.vector.tensor_copy(bcs, bc)
                nc.gpsimd.tensor_mul(og_n[:, pss_n, :], og_n[:, pss_n, :],
                                     bcs)

        def emit_attn(p, stt, interleave=None):
            """Software-pipelined attention stream for parity p."""
            qa, qb, kb, vsb = QA[p], QB[p], KB[p], V_sb[p]
            inter = iter(interleave) if interleave is not None else iter(())
            if nparity == 2:
                for kt in range(KT):
                    nc.sync.dma_start(x_sb[p][:, kt, :], xT_t[:, kt, :])

            n_slots = ST * 2 * 4            # unit slots (kt%4==1)
            n_units = 3 * ST if interleave is not None else 0
            cnt = {"slot": 0, "emit": 0}

            for qt in range(ST):
                qsl = QSL[qt]
                og = og_ring[qt % 2]
                for pss, qtile in ((0, qa), (1, qb)):
                    blk = qt * 2 + pss
                    for kt in range(NKT):
                        ksl = slice(kt * 128, (kt + 1) * 128)
                        # 1) scores + exp for this step
                        sc = ps_sc.tile([128, 2, 512], f32, name="sc",
                                        tag="sc")
                        nc.tensor.matmul(
                            sc[:, 0, :], lhsT=kb[0:64, ksl],
                            rhs=qtile[0:64, qsl], start=True, stop=True)
                        nc.tensor.matmul(
                            sc[:, 1, :], lhsT=kb[64:128, ksl],
                            rhs=qtile[64:128, qsl], start=True, stop=True)
                        pt = ptp.tile([128, 2, 512], bf16, name="pt",
                                      tag="pt")
                        nc.scalar.activation(pt, sc, Exp, scale=0.125)
                        # 2) flush the P*V + extras queued two steps ago --
                        # the two-step lag hides the acc-drain DVE latency
                        # at block boundaries behind two exp periods
                        while len(stt["drq"]) >= 3:
                            stt["drq"].pop(0)()
                        # 3) queue this step's P*V + extras

                        def mk(qt=qt, pss=pss, kt=kt, pt=pt, og=og, qsl=qsl,
                               blk=blk):
                            def go():
                                if kt == 0:
                                    stt["accA"] = ps_acc.tile(
                                        [128, 512], f32, name="accA",
                                        tag="accA")
                                    stt["accB"] = ps_acc.tile(
                                        [128, 512], f32, name="accB",
                                        tag="accB")
                                accA, accB = stt["accA"], stt["accB"]
                                nc.tensor.matmul(
                                    accA[0:65, :], lhsT=vsb[:, kt, 0:65],
                                    rhs=pt[:, 0, :], start=(kt == 0),
                                    stop=(kt == NKT - 1))
                                nc.tensor.matmul(
                                    accB[0:65, :], lhsT=vsb[:, kt, 0:65],
                                    rhs=pt[:, 1, :], start=(kt == 0),
                                    stop=(kt == NKT - 1))
                                if kt == 0:
                                    flush_norm(stt)
                                if kt % 4 == 3 and stt["oproj"]:
                                    og_o, qsl_o, mt_o = stt["oproj"].pop(0)
                                    emit_oproj(og_o, qsl_o, mt_o)
                                if kt % 4 == 1:
                                    cnt["slot"] += 1
                                    if (cnt["emit"] * n_slots
                                            < cnt["slot"] * n_units):
                                        u = next(inter, None)
                                        if u is not None:
                                            u()
                                            cnt["emit"] += 1
                                if kt == NKT - 1:
                                    # drain unnormalized O^T + denominators
                                    nc.vector.tensor_copy(og[0:64, pss, :],
                                                          accA[0:64, :])
                                    nc.vector.tensor_copy(og[64:128, pss, :],
                                                          accB[0:64, :])
                                    dgq = dgq_ring[blk % 2]
                                    nc.vector.tensor_copy(dgq[0:1, :],
                                                          accA[64:65, :])
                                    tmpd = oxp.tile([1, 512], f32,
                                                    name="tmpd", tag="tmpd")
                                    nc.vector.tensor_copy(tmpd,
                                                          accB[64:65, :])
                                    nc.sync.dma_start(dgq[1:2, :], tmpd)
                                    stt["norm"].append((og, pss, dgq))
                                    if pss == 1:
                                        stt["oproj"].extend(
                                            (og, qsl, mt) for mt in range(KT))
                            return go
                        stt["drq"].append(mk())
            # leftover proj units (normally none)
            for u in inter:
                u()

        def drain_carry(stt):
            """Emit everything still pending (ends one logical iteration)."""
            while stt["drq"]:
                stt["drq"].pop(0)()
            flush_norm(stt)
            while stt["oproj"]:
                og_o, qsl_o, mt_o = stt["oproj"].pop(0)
                emit_oproj(og_o, qsl_o, mt_o)

        # ---------- program ----------
        emit_proj_block(0)
        if repeat == 1:
            stt = {"drq": [], "norm": [], "oproj": []}
            emit_attn(0, stt)
            drain_carry(stt)
        else:
            def carried_state():
                return {
                    "drq": [],
                    "norm": [(og_ring[1], 1, dgq_ring[1])],
                    "oproj": [(og_ring[1], QSL[3], mt) for mt in range(KT)],
                }
            with tc.For_i(0, repeat // 2, 1,
                          hint_engines=(mybir.EngineType.PE,
                                        mybir.EngineType.Activation,
                                        mybir.EngineType.DVE,
                                        mybir.EngineType.SP),
                          staggered_reset=True):
                stt = carried_state()
                emit_attn(0, stt, interleave=proj_units(1))
                emit_attn(1, stt, interleave=proj_units(0))
                while stt["drq"]:
                    stt["drq"].pop(0)()
            # epilogue: finish the last iteration's carried work
            stt = carried_state()
            drain_carry(stt)

    nc.finalize()
    return nc


def _rope_tables(relative_positions):
    """cos/sin tables [64, S] in the permuted per-head layout."""
    rp = np.asarray(relative_positions, dtype=np.float32)
    half = HD // 2
    inv = (1.0 / (THETA ** (np.arange(0, half, 2, dtype=np.float32) / half)))
    fx = rp[:, 0:1] * inv[None, :]          # [S, 16]
    fy = rp[:, 1:2] * inv[None, :]          # [S, 16]
    F = np.concatenate([fx, fy, fx, fy], axis=1).T.astype(np.float32)  # [64,S]
    cos = np.cos(F).astype(np.float32)
    sin = np.sin(F).astype(np.float32)
    sin[0:32] = -sin[0:32]                  # even rows get -sin
    return cos, sin


def _bf16(a):
    return np.ascontiguousarray(np.asarray(a, np.float32).astype(
        ml_dtypes.bfloat16))


def _make_in_maps(x, relative_positions, Wq, Wk, Wv, Wo):
    x = np.asarray(x, np.float32)
    Wq = np.asarray(Wq, np.float32)
    Wk = np.asarray(Wk, np.float32)
    Wv = np.asarray(Wv, np.float32)
    Wo = np.asarray(Wo, np.float32)
    cos, sin = _rope_tables(relative_positions)
    tqc = _bf16(np.vstack([cos, cos]))
    tqs = _bf16(np.vstack([sin, sin]))
    tkc, tks = _bf16(cos), _bf16(sin)
    xTb = [_bf16(x[b].T) for b in range(B)]
    permm = _bf16(_PERMM)

    in_maps = []
    for core in range(NCORE):
        b, g = divmod(core, HKV)
        heads = [4 * g + j for j in range(4)]
        wq_p = np.concatenate(
            [Wq[:, 64 * h + PERM64] for h in heads], axis=1)      # [DM, 256]
        wkv_p = np.concatenate(
            [Wk[:, 64 * g + PERM64], Wv[:, 64 * g:64 * g + 64]], axis=1)
        wo_g = Wo[256 * g:256 * (g + 1), :]
        in_maps.append({
            "xT": xTb[b],
            "wq": _bf16(wq_p),
            "wkv": _bf16(wkv_p),
            "wo": _bf16(wo_g),
            "tqc": tqc, "tqs": tqs, "tkc": tkc, "tks": tks,
            "seld": _SEL, "permd": permm,
        })
    return in_maps


def _run(nc, in_maps):
    from concourse.bass_utils import run_bass_kernel_spmd
    last_err = None
    for _ in range(3):
        try:
            return run_bass_kernel_spmd(nc, in_maps, list(range(NCORE)))
        except Exception as e:  # transient NRT device errors happen
            last_err = e
    raise last_err


def kernel(x, relative_positions, Wq, Wk, Wv, Wo):
    if "p1" not in _prog_cache:
        _prog_cache["p1"] = _build_program(1)
    nc = _prog_cache["p1"]
    in_maps = _make_in_maps(x, relative_positions, Wq, Wk, Wv, Wo)
    res = _run(nc, in_maps)
    y = np.zeros((B, S, DM), np.float32)
    for core in range(NCORE):
        b = core // HKV
        y[b] += res.results[core]["yT"].T
    return y
